# revision 31
# baseline (speedup 1.0000x reference)
"""Trainium2 Bass kernel for the BlockDiagonalACDC layer (parity-split L1).

out = riffle(idct2(gconv(dct2(gconv(x, A)), D))) + bias, all linear along
the feature dim (4096). DCT-II parity symmetry halves both dense passes:
  fwd:  u± = z1[:2048] ± reverse(z1[2048:]); z2_even = u+ @ E1,
        z2_odd = u- @ O1  (E1/O1 = parity column slices of Ct[:2048]).
        The reversal is free: groups >= 16 of gconv(A) run with a
        free-dim-reversed lhsT so their PSUM comes out partition-reversed.
  gconv(D): conjugated into parity-block space (block-diag quadrant lhsT
        tiles built on device from D).
  inv:  s = z3_even @ GmE, t = z3_odd @ GmO with columns pre-permuted so
        riffle+reversal land as contiguous output runs; out = s±t, bias
        injected as half-sum/diff rows via K=1 matmuls into PSUM.

Sharding: pure data parallel, 2048 batch rows per core on 8 cores.
"""

import numpy as np
import ml_dtypes

import concourse.bacc as bacc
import concourse.mybir as mybir
from concourse.tile import TileContext
from concourse.bass_utils import run_bass_kernel_spmd
from concourse.masks import make_identity

N_BATCH, D_FEAT, GROUPS = 16384, 4096, 32
N_CORES = 8
N_SHARD = N_BATCH // N_CORES      # 2048 rows per core
CHUNK = 512                       # batch rows per pipeline chunk
N_CHUNKS = N_SHARD // CHUNK       # 4
FTILES = D_FEAT // 128            # 32
HT = FTILES // 2                  # 16
QW = 512                          # inverse strip width
NQ = 2048 // QW                   # 4 strips

_BF16 = mybir.dt.bfloat16
_F32 = mybir.dt.float32

# output runs per inverse strip c (args q in [QW*c, QW*(c+1))):
#   s+t -> ascending from PLUS_START[c]; s-t -> reversed into MINUS_LO[c]..
PLUS_START = [QW * c if c < NQ // 2 else 1024 + QW * c for c in range(NQ)]
MINUS_LO = [(4096 - QW) - QW * c if c < NQ // 2 else (3072 - QW) - QW * c
            for c in range(NQ)]


def _host_constants():
    N = D_FEAT
    H = N // 2
    j = np.arange(N, dtype=np.float64)
    k = np.arange(N, dtype=np.float64)[:, None]
    ang = np.pi * k * (2.0 * j[None, :] + 1.0) / (2.0 * N)
    C = 2.0 * np.cos(ang)
    Ct = np.ascontiguousarray(C.T)                    # [j, k]
    w = np.ones(N); w[0] = 0.5
    Gm = (1.0 / N) * w[:, None] * np.cos(ang)         # [k, j]

    E1 = Ct[:H, 0::2]      # [2048, 2048]
    O1 = Ct[:H, 1::2]

    def tile_fwd(M):       # [tau, p, fc, m]
        return np.ascontiguousarray(
            M.reshape(HT, 128, HT, 128).transpose(2, 1, 0, 3))
    fwd_host = np.stack([tile_fwd(E1), tile_fwd(O1)]).astype(ml_dtypes.bfloat16)

    cols = np.concatenate([np.arange(0, H, 2), np.arange(1, H, 2)])
    GmE = Gm[0::2][:, :H][:, cols]    # [2048 m, 2048 q]
    GmO = Gm[1::2][:, :H][:, cols]

    def tile_inv(M):       # [qc, p, kc, q]
        return np.ascontiguousarray(
            M.reshape(HT, 128, NQ, QW).transpose(2, 1, 0, 3))
    inv_host = np.stack([tile_inv(GmE), tile_inv(GmO)]).astype(ml_dtypes.bfloat16)

    out_plus = np.where(cols % 2 == 0, cols // 2, 2048 + (cols - 1) // 2)
    jm = 4095 - cols
    out_minus = np.where(jm % 2 == 0, jm // 2, 2048 + (jm - 1) // 2)
    return fwd_host, inv_host, out_plus.astype(np.int64), out_minus.astype(np.int64)


def _build_program(reps=1):
    nc = bacc.Bacc()
    # xs pre-transposed on host to [128, 16, 4096] so row-tile loads batch
    xs = nc.dram_tensor("xs", (128, N_SHARD // 128, D_FEAT), _F32,
                        kind="ExternalInput")
    # A/D pre-transposed to [128, g, 128] so the full weight loads in one DMA
    Aw = nc.dram_tensor("Aw", (128, GROUPS, 128), _F32, kind="ExternalInput")
    Dw = nc.dram_tensor("Dw", (128, GROUPS, 128), _F32, kind="ExternalInput")
    bias_s = nc.dram_tensor("bias_s", (1, 2048), _F32, kind="ExternalInput")
    bias_t = nc.dram_tensor("bias_t", (1, 2048), _F32, kind="ExternalInput")
    fwdw = nc.dram_tensor("fwdw", (2, HT, 128, HT, 128), _BF16, kind="ExternalInput")
    invw = nc.dram_tensor("invw", (2, NQ, 128, HT, QW), _BF16, kind="ExternalInput")
    out = nc.dram_tensor("out", (N_SHARD, D_FEAT), _F32, kind="ExternalOutput")

    with TileContext(nc) as tc:
        with (
            tc.tile_pool(name="const", bufs=1) as constp,
            tc.tile_pool(name="stage", bufs=3) as stagep,
            tc.tile_pool(name="xbf", bufs=2) as xbfp,
            tc.tile_pool(name="fwp", bufs=3) as fwp,
            tc.tile_pool(name="ivp", bufs=4) as ivp,
            tc.tile_pool(name="ost", bufs=3) as ostp,
            tc.tile_pool(name="mm_ps", bufs=3, space="PSUM") as mmp,
            tc.tile_pool(name="tp_ps", bufs=2, space="PSUM") as tpp,
            tc.tile_pool(name="st_ps", bufs=3, space="PSUM") as stp,
        ):
            # weight loads first so PE's AT/LT transposes start ASAP
            awbf = stagep.tile([128, D_FEAT], _BF16, tag="stage")
            nc.gpsimd.dma_start(awbf[:], Aw[:])
            dwbf_early = stagep.tile([128, D_FEAT], _BF16, tag="stage")
            nc.gpsimd.dma_start(dwbf_early[:], Dw[:])
            ident = constp.tile([128, 128], _BF16, tag="ident")
            make_identity(nc, ident[:])
            ones1 = constp.tile([1, 128], _BF16, tag="ones1")
            nc.gpsimd.memset(ones1[:], 1.0)
            bs_bf = constp.tile([1, 2048], _BF16, tag="bs")
            bt_bf = constp.tile([1, 2048], _BF16, tag="bt")
            nc.gpsimd.dma_start(bs_bf[:], bias_s[:])
            nc.gpsimd.dma_start(bt_bf[:], bias_t[:])

            # ---- A weights: AT[g] = A[g].T
            AT = constp.tile([128, D_FEAT], _BF16, tag="AT")
            for g4 in range(GROUPS // 4):
                ps = tpp.tile([128, 512], _BF16, tag="tp")
                for gg in range(4):
                    g = g4 * 4 + gg
                    nc.tensor.transpose(
                        ps[:, gg * 128:(gg + 1) * 128],
                        awbf[:, g * 128:(g + 1) * 128], ident[:])
                for gg in range(4):
                    g = g4 * 4 + gg
                    sl = slice(g * 128, (g + 1) * 128)
                    psl = slice(gg * 128, (gg + 1) * 128)
                    ceng = (nc.vector.tensor_copy, nc.scalar.copy)[gg % 2]
                    if g < 16:
                        ceng(AT[:, sl], ps[:, psl])
                    else:
                        # store columns reversed: gconvA for groups >= 16
                        # then emits partition-reversed (butterfly-ready) tiles
                        ceng(AT[:, sl],
                             ps[:, (gg + 1) * 128 - 1:gg * 128 - 1 if gg else None:-1])

            # ---- D weights: conjugated quadrant tiles LT[x][y]
            dwbf = dwbf_early
            LT = [[constp.tile([128, HT * 128], _BF16, tag=f"LT{x}{y}",
                                name=f"LT{x}{y}")
                   for y in range(2)] for x in range(2)]
            for x in range(2):
                for y in range(2):
                    nc.gpsimd.memset(LT[x][y][:], 0.0)
            for tau in range(HT):
                g1 = 2 * tau
                for x in range(2):
                    ps = tpp.tile([128, 512], _BF16, tag="tp")
                    nc.tensor.transpose(
                        ps[:, 0:128],
                        dwbf[:, g1 * 128 + x:(g1 + 2) * 128:2],
                        ident[:])
                    for y in range(2):
                        ca = (nc.vector.tensor_copy, nc.scalar.copy)[y]
                        cb = (nc.scalar.copy, nc.vector.tensor_copy)[y]
                        ca(LT[x][y][0:64, tau * 128:tau * 128 + 64],
                           ps[0:64, y:128:2])
                        cb(LT[x][y][64:128, tau * 128 + 64:(tau + 1) * 128],
                           ps[64:128, y:128:2])

            rep_ctx = tc.For_i(0, reps, 1) if reps > 1 else None
            if rep_ctx is not None:
                rep_ctx.__enter__()
            for ci in range(N_CHUNKS):
                r0 = ci * CHUNK
                # ---- transpose-in
                xT = stagep.tile([128, FTILES * CHUNK], _BF16, tag="stage")
                nt0 = r0 // 128
                for ntp in range(2):
                    xbfs = []
                    for nn in range(2):
                        xbf = xbfp.tile([128, D_FEAT], _BF16, tag="xbf")
                        nc.gpsimd.dma_start(
                            xbf[:], xs[:, nt0 + ntp * 2 + nn, :])
                        xbfs.append(xbf)
                    for fc in range(FTILES):
                        ps = tpp.tile([128, 512], _BF16, tag="tp")
                        for nn in range(2):
                            nc.tensor.transpose(
                                ps[:, nn * 128:(nn + 1) * 128],
                                xbfs[nn][:, fc * 128:(fc + 1) * 128], ident[:])
                        eng = nc.vector if fc % 2 else nc.scalar
                        (eng.tensor_copy if eng is nc.vector else eng.copy)(
                            xT[:, fc * CHUNK + ntp * 256:
                               fc * CHUNK + ntp * 256 + 256],
                            ps[:, 0:256])
                # ---- gconvA; groups >= 16 with reversed lhsT -> stored
                # at slot 47-g as reversed tiles (butterfly-ready)
                z1 = stagep.tile([128, FTILES * CHUNK], _BF16, tag="stage")
                for g in range(GROUPS):
                    ps = mmp.tile([128, CHUNK], _F32, tag="mm")
                    nc.tensor.matmul(
                        ps[:], AT[:, g * 128:(g + 1) * 128],
                        xT[:, g * CHUNK:(g + 1) * CHUNK],
                        start=True, stop=True)
                    slot = g if g < 16 else 47 - g
                    if g % 2:
                        nc.scalar.copy(z1[:, slot * CHUNK:(slot + 1) * CHUNK], ps[:])
                    else:
                        nc.vector.tensor_copy(
                            z1[:, slot * CHUNK:(slot + 1) * CHUNK], ps[:])

                # ---- butterfly: uu = [up tiles 0..15 | um tiles 16..31]
                uu = stagep.tile([128, FTILES * CHUNK], _BF16, tag="stage")
                for t in range(HT):
                    lo = slice(t * CHUNK, (t + 1) * CHUNK)
                    hi = slice((16 + t) * CHUNK, (17 + t) * CHUNK)
                    nc.vector.tensor_add(uu[:, lo], z1[:, lo], z1[:, hi])
                    nc.vector.tensor_sub(uu[:, hi], z1[:, lo], z1[:, hi])

                # ---- fwd dense: z2 = [E-block | O-block]
                z2 = stagep.tile([128, FTILES * CHUNK], _BF16, tag="stage")
                for b in range(2):
                    for tau in range(HT):
                        fw = fwp.tile([128, HT, 128], _BF16, tag="fw")
                        nc.sync.dma_start(fw[:], fwdw[b, tau])
                        ps = mmp.tile([128, CHUNK], _F32, tag="mm")
                        for fc in range(HT):
                            nc.tensor.matmul(
                                ps[:], fw[:, fc, :],
                                uu[:, (16 * b + fc) * CHUNK:
                                   (16 * b + fc + 1) * CHUNK],
                                start=(fc == 0), stop=(fc == HT - 1))
                        slot = 16 * b + tau
                        if tau % 2:
                            nc.scalar.copy(
                                z2[:, slot * CHUNK:(slot + 1) * CHUNK], ps[:])
                        else:
                            nc.vector.tensor_copy(
                                z2[:, slot * CHUNK:(slot + 1) * CHUNK], ps[:])

                # ---- conjugated gconvD
                z3 = stagep.tile([128, FTILES * CHUNK], _BF16, tag="stage")
                for y in range(2):
                    for tau in range(HT):
                        ps = mmp.tile([128, CHUNK], _F32, tag="mm")
                        for x in range(2):
                            nc.tensor.matmul(
                                ps[:], LT[x][y][:, tau * 128:(tau + 1) * 128],
                                z2[:, (16 * x + tau) * CHUNK:
                                   (16 * x + tau + 1) * CHUNK],
                                start=(x == 0), stop=(x == 1))
                        slot = 16 * y + tau
                        if tau % 2:
                            nc.scalar.copy(
                                z3[:, slot * CHUNK:(slot + 1) * CHUNK], ps[:])
                        else:
                            nc.vector.tensor_copy(
                                z3[:, slot * CHUNK:(slot + 1) * CHUNK], ps[:])

                # ---- inverse strips + butterfly + bias + store
                for qc in range(NQ):
                    ivs = []
                    for b in range(2):
                        for h in range(2):
                            iv = ivp.tile([128, HT // 2, QW], _BF16, tag="iv")
                            nc.sync.dma_start(
                                iv[:], invw[b, qc, :, h * (HT // 2):(h + 1) * (HT // 2)])
                            ivs.append(iv)
                    for nt in range(CHUNK // 128):
                        pst = []
                        for b in range(2):
                            ps = stp.tile([128, QW], _F32, tag="st")
                            for kc in range(HT):
                                nc.tensor.matmul(
                                    ps[:],
                                    z3[:, (16 * b + kc) * CHUNK + nt * 128:
                                       (16 * b + kc) * CHUNK + (nt + 1) * 128],
                                    ivs[2 * b + kc // 8][:, kc % 8, :],
                                    start=(kc == 0), stop=False)
                            brow = bs_bf if b == 0 else bt_bf
                            nc.tensor.matmul(
                                ps[:], ones1[:],
                                brow[0:1, qc * QW:(qc + 1) * QW],
                                start=False, stop=True)
                            pst.append(ps)
                        tsb = ostp.tile([128, QW], _F32, tag="ost")
                        nc.scalar.copy(tsb[:], pst[1][:])
                        op = ostp.tile([128, QW], _F32, tag="ost")
                        om = ostp.tile([128, QW], _F32, tag="ost")
                        nc.vector.tensor_add(op[:], pst[0][:], tsb[:])
                        nc.vector.tensor_sub(om[:, ::-1], pst[0][:], tsb[:])
                        rows = slice(r0 + nt * 128, r0 + (nt + 1) * 128)
                        nc.sync.dma_start(
                            out[rows, PLUS_START[qc]:PLUS_START[qc] + QW], op[:])
                        nc.sync.dma_start(
                            out[rows, MINUS_LO[qc]:MINUS_LO[qc] + QW],
                            om[:])
            if rep_ctx is not None:
                rep_ctx.__exit__(None, None, None)
    nc.finalize()
    return nc


_CACHE = {}


def kernel(x, A, D, bias):
    if "nc" not in _CACHE:
        _CACHE["consts"] = _host_constants()
        _CACHE["nc"] = _build_program()
    nc = _CACHE["nc"]
    fwd_host, inv_host, out_plus, out_minus = _CACHE["consts"]

    bias_v = np.asarray(bias, dtype=np.float64).reshape(-1)
    bs = ((bias_v[out_plus] + bias_v[out_minus]) / 2).astype(np.float32)[None]
    bt = ((bias_v[out_plus] - bias_v[out_minus]) / 2).astype(np.float32)[None]

    x = np.ascontiguousarray(x, dtype=np.float32)
    At = np.ascontiguousarray(
        np.asarray(A, dtype=np.float32).transpose(1, 0, 2))
    Dt = np.ascontiguousarray(
        np.asarray(D, dtype=np.float32).transpose(1, 0, 2))
    in_maps = []
    for c in range(N_CORES):
        shard = x[c * N_SHARD:(c + 1) * N_SHARD]
        xs_t = np.ascontiguousarray(
            shard.reshape(N_SHARD // 128, 128, D_FEAT).transpose(1, 0, 2))
        in_maps.append({
            "xs": xs_t,
            "Aw": At, "Dw": Dt,
            "bias_s": bs, "bias_t": bt,
            "fwdw": fwd_host, "invw": inv_host,
        })
    res = run_bass_kernel_spmd(nc, in_maps, core_ids=list(range(N_CORES)))
    return np.concatenate([res.results[c]["out"] for c in range(N_CORES)], axis=0)



# revision 35
# speedup vs baseline: 1.0130x; 1.0130x over previous
"""Trainium2 Bass kernel for the BlockDiagonalACDC layer (parity-split L1).

out = riffle(idct2(gconv(dct2(gconv(x, A)), D))) + bias, all linear along
the feature dim (4096). DCT-II parity symmetry halves both dense passes:
  fwd:  u± = z1[:2048] ± reverse(z1[2048:]); z2_even = u+ @ E1,
        z2_odd = u- @ O1  (E1/O1 = parity column slices of Ct[:2048]).
        The reversal is free: groups >= 16 of gconv(A) run with a
        free-dim-reversed lhsT so their PSUM comes out partition-reversed.
  gconv(D): conjugated into parity-block space (block-diag quadrant lhsT
        tiles built on device from D).
  inv:  s = z3_even @ GmE, t = z3_odd @ GmO with columns pre-permuted so
        riffle+reversal land as contiguous output runs; out = s±t, bias
        injected as half-sum/diff rows via K=1 matmuls into PSUM.

Sharding: pure data parallel, 2048 batch rows per core on 8 cores.
"""

import numpy as np
import ml_dtypes

import concourse.bacc as bacc
import concourse.mybir as mybir
from concourse.tile import TileContext
from concourse.bass_utils import run_bass_kernel_spmd
from concourse.masks import make_identity

N_BATCH, D_FEAT, GROUPS = 16384, 4096, 32
N_CORES = 8
N_SHARD = N_BATCH // N_CORES      # 2048 rows per core
CHUNK = 512                       # batch rows per pipeline chunk
N_CHUNKS = N_SHARD // CHUNK       # 4
FTILES = D_FEAT // 128            # 32
HT = FTILES // 2                  # 16
QW = 512                          # inverse strip width
NQ = 2048 // QW                   # 4 strips

_BF16 = mybir.dt.bfloat16
_F32 = mybir.dt.float32

# output runs per inverse strip c (args q in [QW*c, QW*(c+1))):
#   s+t -> ascending from PLUS_START[c]; s-t -> reversed into MINUS_LO[c]..
PLUS_START = [QW * c if c < NQ // 2 else 1024 + QW * c for c in range(NQ)]
MINUS_LO = [(4096 - QW) - QW * c if c < NQ // 2 else (3072 - QW) - QW * c
            for c in range(NQ)]


def _host_constants():
    N = D_FEAT
    H = N // 2
    j = np.arange(N, dtype=np.float64)
    k = np.arange(N, dtype=np.float64)[:, None]
    ang = np.pi * k * (2.0 * j[None, :] + 1.0) / (2.0 * N)
    C = 2.0 * np.cos(ang)
    Ct = np.ascontiguousarray(C.T)                    # [j, k]
    w = np.ones(N); w[0] = 0.5
    Gm = (1.0 / N) * w[:, None] * np.cos(ang)         # [k, j]

    E1 = Ct[:H, 0::2]      # [2048, 2048]
    O1 = Ct[:H, 1::2]

    def tile_fwd(M):       # [tau, p, fc, m]
        return np.ascontiguousarray(
            M.reshape(HT, 128, HT, 128).transpose(2, 1, 0, 3))
    fwd_host = np.stack([tile_fwd(E1), tile_fwd(O1)]).astype(ml_dtypes.bfloat16)

    cols = np.concatenate([np.arange(0, H, 2), np.arange(1, H, 2)])
    GmE = Gm[0::2][:, :H][:, cols]    # [2048 m, 2048 q]
    GmO = Gm[1::2][:, :H][:, cols]

    def tile_inv(M):       # [qc, p, kc, q]
        return np.ascontiguousarray(
            M.reshape(HT, 128, NQ, QW).transpose(2, 1, 0, 3))
    inv_host = np.stack([tile_inv(GmE), tile_inv(GmO)]).astype(ml_dtypes.bfloat16)

    out_plus = np.where(cols % 2 == 0, cols // 2, 2048 + (cols - 1) // 2)
    jm = 4095 - cols
    out_minus = np.where(jm % 2 == 0, jm // 2, 2048 + (jm - 1) // 2)
    return fwd_host, inv_host, out_plus.astype(np.int64), out_minus.astype(np.int64)


def _build_program(reps=1):
    nc = bacc.Bacc()
    # xs pre-transposed on host to [128, 16, 4096] so row-tile loads batch
    xs = nc.dram_tensor("xs", (128, N_SHARD // 128, D_FEAT), _F32,
                        kind="ExternalInput")
    # A/D pre-transposed to [128, g, 128] so the full weight loads in one DMA
    Aw = nc.dram_tensor("Aw", (128, GROUPS, 128), _F32, kind="ExternalInput")
    Dw = nc.dram_tensor("Dw", (128, GROUPS, 128), _F32, kind="ExternalInput")
    bias_s = nc.dram_tensor("bias_s", (1, 2048), _F32, kind="ExternalInput")
    bias_t = nc.dram_tensor("bias_t", (1, 2048), _F32, kind="ExternalInput")
    fwdw = nc.dram_tensor("fwdw", (2, HT, 128, HT, 128), _BF16, kind="ExternalInput")
    invw = nc.dram_tensor("invw", (2, NQ, 128, HT, QW), _BF16, kind="ExternalInput")
    out = nc.dram_tensor("out", (N_SHARD, D_FEAT), _F32, kind="ExternalOutput")

    with TileContext(nc) as tc:
        with (
            tc.tile_pool(name="const", bufs=1) as constp,
            tc.tile_pool(name="stage", bufs=3) as stagep,
            tc.tile_pool(name="xbf", bufs=2) as xbfp,
            tc.tile_pool(name="fwp", bufs=3) as fwp,
            tc.tile_pool(name="ivp", bufs=4) as ivp,
            tc.tile_pool(name="ost", bufs=3) as ostp,
            tc.tile_pool(name="mm_ps", bufs=3, space="PSUM") as mmp,
            tc.tile_pool(name="tp_ps", bufs=2, space="PSUM") as tpp,
            tc.tile_pool(name="st_ps", bufs=3, space="PSUM") as stp,
        ):
            # weight loads first so PE's AT/LT transposes start ASAP
            awbf = stagep.tile([128, D_FEAT], _BF16, tag="stage")
            nc.gpsimd.dma_start(awbf[:], Aw[:])
            dwbf_early = stagep.tile([128, D_FEAT], _BF16, tag="stage")
            nc.gpsimd.dma_start(dwbf_early[:], Dw[:])
            ident = constp.tile([128, 128], _BF16, tag="ident")
            make_identity(nc, ident[:])
            ones1 = constp.tile([1, 128], _BF16, tag="ones1")
            nc.gpsimd.memset(ones1[:], 1.0)
            bs_bf = constp.tile([1, 2048], _BF16, tag="bs")
            bt_bf = constp.tile([1, 2048], _BF16, tag="bt")
            nc.gpsimd.dma_start(bs_bf[:], bias_s[:])
            nc.gpsimd.dma_start(bt_bf[:], bias_t[:])

            # ---- A weights: AT[g] = A[g].T
            AT = constp.tile([128, D_FEAT], _BF16, tag="AT")
            for g4 in range(GROUPS // 4):
                # setup transposes run through mmp (3 bufs, idle during
                # setup) so PE isn't throttled by tpp's 2-buf depth
                ps = mmp.tile([128, 512], _BF16, tag="mm")
                for gg in range(4):
                    g = g4 * 4 + gg
                    nc.tensor.transpose(
                        ps[:, gg * 128:(gg + 1) * 128],
                        awbf[:, g * 128:(g + 1) * 128], ident[:])
                for gg in range(4):
                    g = g4 * 4 + gg
                    sl = slice(g * 128, (g + 1) * 128)
                    psl = slice(gg * 128, (gg + 1) * 128)
                    ceng = (nc.vector.tensor_copy, nc.scalar.copy)[gg % 2]
                    if g < 16:
                        ceng(AT[:, sl], ps[:, psl])
                    else:
                        # store columns reversed: gconvA for groups >= 16
                        # then emits partition-reversed (butterfly-ready) tiles
                        ceng(AT[:, sl],
                             ps[:, (gg + 1) * 128 - 1:gg * 128 - 1 if gg else None:-1])

            # ---- D weights: conjugated quadrant tiles LT[x][y]
            dwbf = dwbf_early
            LT = [[constp.tile([128, HT * 128], _BF16, tag=f"LT{x}{y}",
                                name=f"LT{x}{y}")
                   for y in range(2)] for x in range(2)]
            for x in range(2):
                for y in range(2):
                    nc.gpsimd.memset(LT[x][y][:], 0.0)
            for tau in range(HT):
                g1 = 2 * tau
                for x in range(2):
                    ps = mmp.tile([128, 512], _BF16, tag="mm")
                    nc.tensor.transpose(
                        ps[:, 0:128],
                        dwbf[:, g1 * 128 + x:(g1 + 2) * 128:2],
                        ident[:])
                    for y in range(2):
                        ca = (nc.vector.tensor_copy, nc.scalar.copy)[y]
                        cb = (nc.scalar.copy, nc.vector.tensor_copy)[y]
                        ca(LT[x][y][0:64, tau * 128:tau * 128 + 64],
                           ps[0:64, y:128:2])
                        cb(LT[x][y][64:128, tau * 128 + 64:(tau + 1) * 128],
                           ps[64:128, y:128:2])

            rep_ctx = tc.For_i(0, reps, 1) if reps > 1 else None
            if rep_ctx is not None:
                rep_ctx.__enter__()
            for ci in range(N_CHUNKS):
                r0 = ci * CHUNK
                # ---- transpose-in
                xT = stagep.tile([128, FTILES * CHUNK], _BF16, tag="stage")
                nt0 = r0 // 128
                for ntp in range(2):
                    xbfs = []
                    for nn in range(2):
                        xbf = xbfp.tile([128, D_FEAT], _BF16, tag="xbf")
                        nc.gpsimd.dma_start(
                            xbf[:], xs[:, nt0 + ntp * 2 + nn, :])
                        xbfs.append(xbf)
                    for fc in range(FTILES):
                        ps = tpp.tile([128, 512], _BF16, tag="tp")
                        for nn in range(2):
                            nc.tensor.transpose(
                                ps[:, nn * 128:(nn + 1) * 128],
                                xbfs[nn][:, fc * 128:(fc + 1) * 128], ident[:])
                        eng = nc.vector if fc % 2 else nc.scalar
                        (eng.tensor_copy if eng is nc.vector else eng.copy)(
                            xT[:, fc * CHUNK + ntp * 256:
                               fc * CHUNK + ntp * 256 + 256],
                            ps[:, 0:256])
                # ---- gconvA; groups >= 16 with reversed lhsT -> stored
                # at slot 47-g as reversed tiles (butterfly-ready)
                z1 = stagep.tile([128, FTILES * CHUNK], _BF16, tag="stage")
                for g in range(GROUPS):
                    ps = mmp.tile([128, CHUNK], _F32, tag="mm")
                    nc.tensor.matmul(
                        ps[:], AT[:, g * 128:(g + 1) * 128],
                        xT[:, g * CHUNK:(g + 1) * CHUNK],
                        start=True, stop=True)
                    slot = g if g < 16 else 47 - g
                    if g % 2:
                        nc.scalar.copy(z1[:, slot * CHUNK:(slot + 1) * CHUNK], ps[:])
                    else:
                        nc.vector.tensor_copy(
                            z1[:, slot * CHUNK:(slot + 1) * CHUNK], ps[:])

                # ---- butterfly: uu = [up tiles 0..15 | um tiles 16..31]
                uu = stagep.tile([128, FTILES * CHUNK], _BF16, tag="stage")
                for t in range(HT):
                    lo = slice(t * CHUNK, (t + 1) * CHUNK)
                    hi = slice((16 + t) * CHUNK, (17 + t) * CHUNK)
                    nc.vector.tensor_add(uu[:, lo], z1[:, lo], z1[:, hi])
                    nc.vector.tensor_sub(uu[:, hi], z1[:, lo], z1[:, hi])

                # ---- fwd dense: z2 = [E-block | O-block]
                z2 = stagep.tile([128, FTILES * CHUNK], _BF16, tag="stage")
                for b in range(2):
                    for tau in range(HT):
                        fw = fwp.tile([128, HT, 128], _BF16, tag="fw")
                        nc.sync.dma_start(fw[:], fwdw[b, tau])
                        ps = mmp.tile([128, CHUNK], _F32, tag="mm")
                        for fc in range(HT):
                            nc.tensor.matmul(
                                ps[:], fw[:, fc, :],
                                uu[:, (16 * b + fc) * CHUNK:
                                   (16 * b + fc + 1) * CHUNK],
                                start=(fc == 0), stop=(fc == HT - 1))
                        slot = 16 * b + tau
                        if tau % 2:
                            nc.scalar.copy(
                                z2[:, slot * CHUNK:(slot + 1) * CHUNK], ps[:])
                        else:
                            nc.vector.tensor_copy(
                                z2[:, slot * CHUNK:(slot + 1) * CHUNK], ps[:])

                # ---- conjugated gconvD
                z3 = stagep.tile([128, FTILES * CHUNK], _BF16, tag="stage")
                for y in range(2):
                    for tau in range(HT):
                        ps = mmp.tile([128, CHUNK], _F32, tag="mm")
                        for x in range(2):
                            nc.tensor.matmul(
                                ps[:], LT[x][y][:, tau * 128:(tau + 1) * 128],
                                z2[:, (16 * x + tau) * CHUNK:
                                   (16 * x + tau + 1) * CHUNK],
                                start=(x == 0), stop=(x == 1))
                        slot = 16 * y + tau
                        if tau % 2:
                            nc.scalar.copy(
                                z3[:, slot * CHUNK:(slot + 1) * CHUNK], ps[:])
                        else:
                            nc.vector.tensor_copy(
                                z3[:, slot * CHUNK:(slot + 1) * CHUNK], ps[:])

                # ---- inverse strips + butterfly + bias + store
                for qc in range(NQ):
                    ivs = []
                    for b in range(2):
                        for h in range(2):
                            iv = ivp.tile([128, HT // 2, QW], _BF16, tag="iv")
                            nc.sync.dma_start(
                                iv[:], invw[b, qc, :, h * (HT // 2):(h + 1) * (HT // 2)])
                            ivs.append(iv)
                    for nt in range(CHUNK // 128):
                        pst = []
                        for b in range(2):
                            ps = stp.tile([128, QW], _F32, tag="st")
                            for kc in range(HT):
                                nc.tensor.matmul(
                                    ps[:],
                                    z3[:, (16 * b + kc) * CHUNK + nt * 128:
                                       (16 * b + kc) * CHUNK + (nt + 1) * 128],
                                    ivs[2 * b + kc // 8][:, kc % 8, :],
                                    start=(kc == 0), stop=False)
                            brow = bs_bf if b == 0 else bt_bf
                            nc.tensor.matmul(
                                ps[:], ones1[:],
                                brow[0:1, qc * QW:(qc + 1) * QW],
                                start=False, stop=True)
                            pst.append(ps)
                        tsb = ostp.tile([128, QW], _F32, tag="ost")
                        nc.scalar.copy(tsb[:], pst[1][:])
                        op = ostp.tile([128, QW], _F32, tag="ost")
                        om = ostp.tile([128, QW], _F32, tag="ost")
                        nc.vector.tensor_add(op[:], pst[0][:], tsb[:])
                        nc.vector.tensor_sub(om[:, ::-1], pst[0][:], tsb[:])
                        rows = slice(r0 + nt * 128, r0 + (nt + 1) * 128)
                        nc.sync.dma_start(
                            out[rows, PLUS_START[qc]:PLUS_START[qc] + QW], op[:])
                        nc.sync.dma_start(
                            out[rows, MINUS_LO[qc]:MINUS_LO[qc] + QW],
                            om[:])
            if rep_ctx is not None:
                rep_ctx.__exit__(None, None, None)
    nc.finalize()
    return nc


_CACHE = {}


def kernel(x, A, D, bias):
    if "nc" not in _CACHE:
        _CACHE["consts"] = _host_constants()
        _CACHE["nc"] = _build_program()
    nc = _CACHE["nc"]
    fwd_host, inv_host, out_plus, out_minus = _CACHE["consts"]

    bias_v = np.asarray(bias, dtype=np.float64).reshape(-1)
    bs = ((bias_v[out_plus] + bias_v[out_minus]) / 2).astype(np.float32)[None]
    bt = ((bias_v[out_plus] - bias_v[out_minus]) / 2).astype(np.float32)[None]

    x = np.ascontiguousarray(x, dtype=np.float32)
    At = np.ascontiguousarray(
        np.asarray(A, dtype=np.float32).transpose(1, 0, 2))
    Dt = np.ascontiguousarray(
        np.asarray(D, dtype=np.float32).transpose(1, 0, 2))
    in_maps = []
    for c in range(N_CORES):
        shard = x[c * N_SHARD:(c + 1) * N_SHARD]
        xs_t = np.ascontiguousarray(
            shard.reshape(N_SHARD // 128, 128, D_FEAT).transpose(1, 0, 2))
        in_maps.append({
            "xs": xs_t,
            "Aw": At, "Dw": Dt,
            "bias_s": bs, "bias_t": bt,
            "fwdw": fwd_host, "invw": inv_host,
        })
    res = run_bass_kernel_spmd(nc, in_maps, core_ids=list(range(N_CORES)))
    return np.concatenate([res.results[c]["out"] for c in range(N_CORES)], axis=0)

